# revision 7
# baseline (speedup 1.0000x reference)
"""HAN (heterogeneous attention network) forward pass on 8 Trainium2 cores.

Row-parallel sharding over the node dimension N=2560 (320 rows/core):
each core computes the full per-head GAT attention for its 320 destination
rows against all 2560 source nodes, the semantic fusion, and its row block
of the DistMult score matrix.  Collectives: one tiny AllReduce (semantic
attention partial sums) and one AllGather of the fused embedding Z.

Core math trick: with leaky-relu slope 0.5 the masked softmax numerator
  exp(lrelu(e_src_i + e_dst_j)) * mask_ij
is computed as Exp(Prelu(e_ij + 1024*(mask_ij - 1), 0.5)): the rank-2
outer sum e_ij and the full-rank mask shift are both produced by the
tensor engine straight into PSUM (the mask shift via a 1024*I identity
matmul with the {-1,0} mask as the moving operand), so the vector engine
touches none of the [N,N] work and the scalar engine does exactly two
passes (Prelu, Exp) per element.  Row sums ride along as a ones-column in
the attention matmul's stationary operand.
"""

import sys
import numpy as np

sys.path.insert(0, "/opt/trn_rl_repo")

import ml_dtypes  # noqa: E402
from concourse import bacc, mybir, tile  # noqa: E402
from concourse.bass_utils import run_bass_kernel_spmd  # noqa: E402

N, F, NHID, H, D, M = 2560, 512, 256, 8, 32, 3
NC = 8
SH = N // NC          # 320 rows per core
JT = N // 128         # 20 j-tiles
FC = F // 128         # 4 feature chunks
BIG = 1024.0
WPC = 33              # per-head column pitch in padded h ([32 dims | ones])
F32, F32R, BF16 = mybir.dt.float32, mybir.dt.float32r, mybir.dt.bfloat16
AF = mybir.ActivationFunctionType
ALU = mybir.AluOpType

_CACHE = {}


def _build_module(trace=False):
    nc = bacc.Bacc("TRN2", target_bir_lowering=False, debug=False, num_devices=NC)

    d_mask = nc.dram_tensor("maskm1", [M, N, SH], BF16, kind="ExternalInput").ap()
    d_ft = nc.dram_tensor("featT", [F, N], F32R, kind="ExternalInput").ap()
    d_ftsh = nc.dram_tensor("featT_sh", [F, SH], F32R, kind="ExternalInput").ap()
    d_wpad = nc.dram_tensor("Wpad", [M, F, H * WPC], F32R, kind="ExternalInput").ap()
    d_aall = nc.dram_tensor("Aall", [F, 2 * M * H], F32R, kind="ExternalInput").ap()
    d_ident = nc.dram_tensor("I1024", [128, 128], BF16, kind="ExternalInput").ap()
    d_ones = nc.dram_tensor("ones_row", [1, N], F32R, kind="ExternalInput").ap()
    d_negbig = nc.dram_tensor("negbig_row", [1, SH], F32R, kind="ExternalInput").ap()
    d_ws = nc.dram_tensor("Wsem", [NHID, NHID], F32R, kind="ExternalInput").ap()
    d_qc = nc.dram_tensor("q_col", [NHID, 1], F32R, kind="ExternalInput").ap()
    d_bsc = nc.dram_tensor("bs_col", [NHID, 1], F32, kind="ExternalInput").ap()
    d_rc = nc.dram_tensor("r_col", [NHID, 1], F32, kind="ExternalInput").ap()
    d_scores = nc.dram_tensor("scores", [SH, N], F32, kind="ExternalOutput").ap()

    with tile.TileContext(nc) as tc:
        with tc.tile_pool(name="const", bufs=1) as cp, \
             tc.tile_pool(name="hbuf", bufs=2) as hp, \
             tc.tile_pool(name="work", bufs=3) as wp, \
             tc.tile_pool(name="small", bufs=2) as sp, \
             tc.tile_pool(name="pst", bufs=3, space="PSUM") as pst, \
             tc.tile_pool(name="pso", bufs=2, space="PSUM") as pso, \
             tc.tile_pool(name="psh", bufs=2, space="PSUM") as psh, \
             tc.tile_pool(name="pse", bufs=1, space="PSUM") as pse, \
             tc.tile_pool(name="dram", bufs=1, space="DRAM") as dp:

            # ---- constants / parameters into SBUF ----
            ft = []
            ftsh = []
            aall = []
            for fc in range(FC):
                t = cp.tile([128, N], F32R, tag=f"ft{fc}")
                nc.sync.dma_start(out=t[:], in_=d_ft[128 * fc:128 * (fc + 1), :])
                ft.append(t)
                t = cp.tile([128, SH], F32R, tag=f"ftsh{fc}")
                nc.sync.dma_start(out=t[:], in_=d_ftsh[128 * fc:128 * (fc + 1), :])
                ftsh.append(t)
                t = cp.tile([128, 2 * M * H], F32R, tag=f"aall{fc}")
                nc.sync.dma_start(out=t[:], in_=d_aall[128 * fc:128 * (fc + 1), :])
                aall.append(t)
            wpad = {}
            for m in range(M):
                for fc in range(FC):
                    t = cp.tile([128, H * WPC], F32R, tag=f"wpad{m}_{fc}")
                    nc.sync.dma_start(
                        out=t[:], in_=d_wpad[m, 128 * fc:128 * (fc + 1), :])
                    wpad[m, fc] = t
            ident = cp.tile([128, 128], BF16, tag="ident")
            nc.sync.dma_start(out=ident[:], in_=d_ident[:])
            ws = {}
            for kc in range(2):
                for oc in range(2):
                    t = cp.tile([128, 128], F32R, tag=f"ws{kc}{oc}")
                    nc.sync.dma_start(
                        out=t[:],
                        in_=d_ws[128 * kc:128 * (kc + 1), 128 * oc:128 * (oc + 1)])
                    ws[kc, oc] = t
            qc, bsc, rc = [], [], []
            for kc in range(2):
                t = cp.tile([128, 1], F32R, tag=f"qc{kc}")
                nc.sync.dma_start(out=t[:], in_=d_qc[128 * kc:128 * (kc + 1), :])
                qc.append(t)
                t = cp.tile([128, 1], F32, tag=f"bsc{kc}")
                nc.sync.dma_start(out=t[:], in_=d_bsc[128 * kc:128 * (kc + 1), :])
                bsc.append(t)
                t = cp.tile([128, 1], F32, tag=f"rc{kc}")
                nc.sync.dma_start(out=t[:], in_=d_rc[128 * kc:128 * (kc + 1), :])
                rc.append(t)

            # ---- phase 1: attention logit row vectors ----
            # Aall columns: [dst rows 0..23 | src rows 24..47].
            # e_dst rows for all nodes staged to DRAM; e_src shard kept in SBUF.
            e_dst_dram = dp.tile([M * H, N], F32R)
            for nb in range(N // 512):
                pe = pse.tile([2 * M * H, 512], F32, tag="pse")
                for fc in range(FC):
                    nc.tensor.matmul(
                        pe[:], aall[fc][:], ft[fc][:, 512 * nb:512 * (nb + 1)],
                        start=(fc == 0), stop=(fc == FC - 1))
                estage = wp.tile([M * H, 512], F32R, tag="estage", bufs=2)
                nc.scalar.activation(estage[:], pe[0:M * H, :], AF.Copy)
                nc.sync.dma_start(
                    out=e_dst_dram[:, 512 * nb:512 * (nb + 1)], in_=estage[:])
            e_sh = cp.tile([2 * M * H, SH], F32R, tag="e_sh")
            pe = pse.tile([2 * M * H, SH], F32, tag="pse")
            for fc in range(FC):
                nc.tensor.matmul(pe[:], aall[fc][:], ftsh[fc][:],
                                 start=(fc == 0), stop=(fc == FC - 1))
            nc.scalar.activation(e_sh[:], pe[:], AF.Copy)

            # fused Z^T chunks per meta-path: zt[m][kc] is [128, SH]
            zt = [[cp.tile([128, SH], F32, tag=f"zt{m}_{kc}",
                           name=f"zt{m}_{kc}") for kc in range(2)]
                  for m in range(M)]

            # ---- phase 2: node-level attention per meta-path ----
            for m in range(M):
                # h for all nodes, padded layout [32 dims | ones] per head
                hm = []
                for jt in range(JT):
                    ph = psh.tile([128, H * WPC], F32, tag="psh")
                    for fc in range(FC):
                        nc.tensor.matmul(
                            ph[:], ft[fc][:, 128 * jt:128 * (jt + 1)],
                            wpad[m, fc][:],
                            start=(fc == 0), stop=(fc == FC - 1))
                    t = hp.tile([128, H * WPC], F32R, tag=f"h{jt}", bufs=1)
                    nc.scalar.activation(t[:], ph[:], AF.Copy)
                    nc.scalar.activation(
                        t[:].rearrange("p (h c) -> p h c", h=H)[:, :, 32:33],
                        ph[:, 0:H].rearrange("p (a b) -> p a b", b=1),
                        AF.Copy, bias=1.0, scale=0.0)
                    hm.append(t)
                # mask tiles for this meta-path (shared across heads)
                mk = []
                for jt in range(JT):
                    t = hp.tile([128, SH], BF16, tag=f"mk{jt}", bufs=1)
                    nc.sync.dma_start(
                        out=t[:], in_=d_mask[m, 128 * jt:128 * (jt + 1), :])
                    mk.append(t)
                for h in range(H):
                    mh = m * H + h
                    lhsT_e = wp.tile([2, N], F32R, tag="lhsTe", bufs=2)
                    nc.sync.dma_start(
                        out=lhsT_e[0:1, :], in_=e_dst_dram[mh:mh + 1, :])
                    nc.sync.dma_start(out=lhsT_e[1:2, :], in_=d_ones[:])
                    rhs_e = wp.tile([2, SH], F32R, tag="rhse", bufs=2)
                    nc.sync.dma_start(out=rhs_e[0:1, :], in_=d_ones[0:1, 0:SH])
                    nc.sync.dma_start(
                        out=rhs_e[1:2, :], in_=e_sh[M * H + mh:M * H + mh + 1, :])
                    po = pso.tile([WPC, SH], F32, tag="pso")
                    for jt in range(JT):
                        pt = pst.tile([128, SH], F32, tag="pst")
                        nc.tensor.matmul(
                            pt[:], lhsT_e[:, 128 * jt:128 * (jt + 1)], rhs_e[:],
                            start=True, stop=False)
                        nc.tensor.matmul(pt[:], ident[:], mk[jt][:],
                                         start=False, stop=True)
                        qt = wp.tile([128, SH], F32, tag="qt")
                        nc.scalar.activation(qt[:], pt[:], AF.Prelu, alpha=0.5)
                        ptile = wp.tile([128, SH], F32R, tag="ptile")
                        nc.scalar.activation(ptile[:], qt[:], AF.Exp)
                        nc.tensor.matmul(
                            po[:], hm[jt][:, WPC * h:WPC * (h + 1)], ptile[:],
                            start=(jt == 0), stop=(jt == JT - 1),
                            skip_group_check=True)
                    # normalize + ELU into Z^T
                    rcp = sp.tile([1, SH], F32, tag="rcp")
                    nc.vector.reciprocal(rcp[:], po[32:33, :])
                    rcpb = sp.tile([32, SH], F32, tag="rcpb")
                    nc.gpsimd.partition_broadcast(rcpb[:], rcp[:])
                    onrm = sp.tile([32, SH], F32, tag="onrm")
                    nc.vector.tensor_tensor(onrm[:], po[0:32, :], rcpb[:], ALU.mult)
                    xmin = sp.tile([32, SH], F32, tag="xmin")
                    nc.vector.tensor_scalar(xmin[:], onrm[:], 0.0, None, ALU.min)
                    emn = sp.tile([32, SH], F32, tag="emn")
                    nc.scalar.activation(emn[:], xmin[:], AF.Exp)
                    xrel = sp.tile([32, SH], F32, tag="xrel")
                    nc.vector.tensor_scalar(xrel[:], onrm[:], 0.0, None, ALU.max)
                    zslice = zt[m][h // 4][32 * (h % 4):32 * (h % 4 + 1), :]
                    nc.vector.tensor_tensor(zslice, xrel[:], emn[:], ALU.add)
                    nc.vector.tensor_scalar(zslice, zslice, -1.0, None, ALU.add)

            # ---- phase 3: semantic attention ----
            ztr = [[cp.tile([128, SH], F32R, tag=f"ztr{m}_{kc}",
                            name=f"ztr{m}_{kc}") for kc in range(2)]
                    for m in range(M)]
            for m in range(M):
                for kc in range(2):
                    nc.scalar.activation(ztr[m][kc][:], zt[m][kc][:], AF.Copy)
            wcat = sp.tile([1, NC], F32, tag="wcat")
            for m in range(M):
                tt = []
                for oc in range(2):
                    pT = pst.tile([128, SH], F32, tag="pst")
                    for kc in range(2):
                        nc.tensor.matmul(
                            pT[:], ws[kc, oc][:], ztr[m][kc][:],
                            start=(kc == 0), stop=(kc == 1))
                    t = wp.tile([128, SH], F32R, tag="tanh")
                    nc.scalar.activation(t[:], pT[:], AF.Tanh, bias=bsc[oc][:])
                    tt.append(t)
                pw = psh.tile([1, SH], F32, tag="psh")
                for oc in range(2):
                    nc.tensor.matmul(pw[:], qc[oc][:], tt[oc][:],
                                     start=(oc == 0), stop=(oc == 1))
                nc.vector.tensor_reduce(wcat[0:1, m:m + 1], pw[:],
                                        mybir.AxisListType.X, ALU.add)
            # AllReduce the 3 partial row sums
            cin = dp.tile([1, NC], F32)
            cout = dp.tile([1, NC], F32, addr_space="Shared")
            nc.sync.dma_start(out=cin[:], in_=wcat[:])
            nc.gpsimd.collective_compute(
                "AllReduce", ALU.add, replica_groups=[list(range(NC))],
                ins=[cin[:].opt()], outs=[cout[:].opt()])
            wsum = sp.tile([1, NC], F32, tag="wsum")
            nc.sync.dma_start(out=wsum[:], in_=cout[:])
            # beta = softmax(wsum[0:3] / N)
            wmean = sp.tile([1, M], F32, tag="wmean")
            nc.vector.tensor_scalar(wmean[:], wsum[0:1, 0:M], 1.0 / N, None,
                                    ALU.mult)
            wmax = sp.tile([1, 1], F32, tag="wmax")
            nc.vector.tensor_reduce(wmax[:], wmean[:], mybir.AxisListType.X,
                                    ALU.max)
            wshift = sp.tile([1, M], F32, tag="wshift")
            nc.vector.tensor_scalar(wshift[:], wmean[:], wmax[:], None,
                                    ALU.subtract)
            wexp = sp.tile([1, M], F32, tag="wexp")
            nc.scalar.activation(wexp[:], wshift[:], AF.Exp)
            wden = sp.tile([1, 1], F32, tag="wden")
            nc.vector.tensor_reduce(wden[:], wexp[:], mybir.AxisListType.X,
                                    ALU.add)
            wrec = sp.tile([1, 1], F32, tag="wrec")
            nc.vector.reciprocal(wrec[:], wden[:])
            beta = sp.tile([1, M], F32, tag="beta")
            nc.vector.tensor_scalar(beta[:], wexp[:], wrec[:], None, ALU.mult)
            bb = []
            for m in range(M):
                t = sp.tile([128, 1], F32, tag=f"bb{m}")
                nc.gpsimd.partition_broadcast(t[:], beta[0:1, m:m + 1])
                bb.append(t)
            # Zf^T = sum_m beta_m * Z_m^T ; also (r * Zf)^T
            zfr, zrl = [], []
            for kc in range(2):
                acc = sp.tile([128, SH], F32, tag=f"zfacc{kc}")
                nc.vector.tensor_scalar(acc[:], zt[0][kc][:], bb[0][:], None,
                                        ALU.mult)
                for m in range(1, M):
                    t = sp.tile([128, SH], F32, tag="zfp")
                    nc.vector.tensor_scalar(t[:], zt[m][kc][:], bb[m][:], None,
                                            ALU.mult)
                    nc.vector.tensor_tensor(acc[:], acc[:], t[:], ALU.add)
                t = cp.tile([128, SH], F32R, tag=f"zfr{kc}")
                nc.scalar.activation(t[:], acc[:], AF.Copy)
                zfr.append(t)
                t2 = sp.tile([128, SH], F32, tag=f"zrl{kc}f32")
                nc.vector.tensor_scalar(t2[:], acc[:], rc[kc][:], None, ALU.mult)
                t3 = cp.tile([128, SH], F32R, tag=f"zrl{kc}")
                nc.scalar.activation(t3[:], t2[:], AF.Copy)
                zrl.append(t3)
            # AllGather Zf^T shards: [256, SH] per core -> [256*NC, SH]
            agin = dp.tile([NHID, SH], F32R)
            agout = dp.tile([NHID * NC, SH], F32R, addr_space="Shared")
            for kc in range(2):
                nc.sync.dma_start(
                    out=agin[128 * kc:128 * (kc + 1), :], in_=zfr[kc][:])
            nc.gpsimd.collective_compute(
                "AllGather", ALU.bypass, replica_groups=[list(range(NC))],
                ins=[agin[:].opt()], outs=[agout[:].opt()])
            # ---- phase 4: DistMult scores ----
            ics = [(0, 128), (128, 128), (256, 64)]
            for nb in range(NC):
                zfull = []
                for kc in range(2):
                    t = wp.tile([128, SH], F32R, tag=f"zfull{kc}")
                    nc.sync.dma_start(
                        out=t[:],
                        in_=agout[NHID * nb + 128 * kc:NHID * nb + 128 * (kc + 1), :])
                    zfull.append(t)
                for ic, (i0, isz) in enumerate(ics):
                    psc = pst.tile([128, SH], F32, tag="pst")
                    for kc in range(2):
                        nc.tensor.matmul(
                            psc[0:isz, :], zrl[kc][:, i0:i0 + isz], zfull[kc][:],
                            start=(kc == 0), stop=(kc == 1))
                    ssb = wp.tile([128, SH], F32, tag="ssb")
                    nc.scalar.activation(ssb[0:isz, :], psc[0:isz, :], AF.Copy)
                    nc.sync.dma_start(
                        out=d_scores[i0:i0 + isz, SH * nb:SH * (nb + 1)],
                        in_=ssb[0:isz, :])

    nc.compile()
    return nc


def _host_prep(features, adjs, W, a_src, a_dst, Ws, bs, q, relations):
    f32 = np.float32
    featT = np.ascontiguousarray(features.T).astype(f32, copy=False)
    # mask - 1 in bf16, transposed so j is the leading (partition) dim
    maskm1 = (adjs > 0).astype(np.float32) - 1.0
    maskm1 = np.ascontiguousarray(maskm1.transpose(0, 2, 1)).astype(
        ml_dtypes.bfloat16)
    Wpad = np.zeros((M, F, H * WPC), dtype=f32)
    for m in range(M):
        for h in range(H):
            Wpad[m, :, WPC * h:WPC * h + D] = W[m, h]
    Aall = np.zeros((F, 2 * M * H), dtype=f32)
    for m in range(M):
        for h in range(H):
            Aall[:, m * H + h] = W[m, h] @ a_dst[m, h]
            Aall[:, M * H + m * H + h] = W[m, h] @ a_src[m, h]
    ident = (np.eye(128, dtype=np.float32) * BIG).astype(ml_dtypes.bfloat16)
    ones_row = np.ones((1, N), dtype=f32)
    negbig_row = np.full((1, SH), -BIG, dtype=f32)
    common = dict(
        featT=featT, Wpad=Wpad, Aall=Aall, I1024=ident, ones_row=ones_row,
        negbig_row=negbig_row, Wsem=np.ascontiguousarray(Ws).astype(f32),
        q_col=np.ascontiguousarray(q.reshape(NHID, 1)).astype(f32),
        bs_col=np.ascontiguousarray(bs.reshape(NHID, 1)).astype(f32),
        r_col=np.ascontiguousarray(relations.reshape(NHID, 1)).astype(f32),
    )
    in_maps = []
    for c in range(NC):
        sl = slice(SH * c, SH * (c + 1))
        in_maps.append(dict(
            common,
            maskm1=np.ascontiguousarray(maskm1[:, :, sl]),
            featT_sh=np.ascontiguousarray(featT[:, sl]),
        ))
    return in_maps


def kernel(features, adjs, W, a_src, a_dst, Ws, bs, q, relations,
           trace=False, **trace_kwargs):
    features = np.asarray(features, dtype=np.float32)
    adjs = np.asarray(adjs)
    in_maps = _host_prep(np.asarray(features, np.float32), np.asarray(adjs),
                         np.asarray(W, np.float32), np.asarray(a_src, np.float32),
                         np.asarray(a_dst, np.float32), np.asarray(Ws, np.float32),
                         np.asarray(bs, np.float32), np.asarray(q, np.float32),
                         np.asarray(relations, np.float32))
    if "mod" not in _CACHE:
        _CACHE["mod"] = _build_module()
    nc = _CACHE["mod"]
    res = run_bass_kernel_spmd(nc, in_maps, list(range(NC)), trace=trace,
                               **trace_kwargs)
    scores = np.concatenate([res.results[c]["scores"] for c in range(NC)], axis=0)
    penalty = np.float32(np.sum(np.asarray(relations, np.float32) ** 2))
    if trace:
        kernel._last_results = res
    return scores, penalty
